# revision 49
# baseline (speedup 1.0000x reference)
"""Trainium2 Bass kernel for coverage-attention (Bahdanau-style with coverage).

Reference computation (per batch row b):
  proj_sum[s,h] = enc_h[b,s,:] @ W_enc[:,h] + (dec_h[b] @ W_dec)[h] + cov[b,s]*w_cov[h]
  e[s]     = sum_h tanh(proj_sum[s,h] + bias) * v[h]
  a        = softmax(where(mask, e, -1e9))
  covloss  = sum_s min(a, cov)
  h_star   = sum_s a[s] * enc_h[b,s,:]
  new_cov  = cov + a

Strategy: data-parallel over batch (64 -> 8 per NeuronCore). Compute in bf16
on the TensorEngine (PSUM accumulates f32). The host pre-shards and
pre-marshals layouts: enc_h is shipped both natural [b,s,e] (for h_star) and
transposed [b,e,s] (for the projection contraction over e), already cast to
bf16. The coverage outer-product cov[s]*w_cov[h] is folded into the
transposed copy on the host (enc' = enc + cov x u with W_enc.T u = w_cov),
and the dec-projection + bias enters as the tanh's per-partition bias, so
the device-side projection is a clean K=128 matmul chain. proj is computed
transposed ([h_part, s_free]) so the v-weighted tanh reduction is a TensorE
matmul. The per-batch emission is software-pipelined: batch b-1's h_star
matmuls hide inside batch b's projection stream, softmax runs unnormalized
(exp straight from the e-score PSUM, normalization folded into the h_star
epilogue), and the attention row is transposed to matmul-column layout via a
small DRAM bounce. A short warm-up matmul burst holds the PE's HAM activity
window open while the first DMAs land.
"""

import sys

sys.path.insert(0, "/opt/trn_rl_repo")

import numpy as np
import ml_dtypes

import concourse.bass as bass
import concourse.mybir as mybir
import concourse.tile as tile
from concourse import bacc
from concourse.bass_utils import run_bass_kernel_spmd

B, S, E, H = 64, 1024, 1024, 512  # E = 2H
NCORES = 8
BL = B // NCORES  # 8 local batch rows per core
KD = 640  # padded contraction dim for dec proj (512 + 1 bias row, padded to 5*128)

BF16 = mybir.dt.bfloat16
F32 = mybir.dt.float32
AF = mybir.ActivationFunctionType
ALU = mybir.AluOpType
AX = mybir.AxisListType

_graphs = {}
LAST_RUN = None  # BassKernelResults of the most recent run (exec_time_ns when traced)


def _build_graph(masked):
    """masked=False is the fast path used when mask is all-ones (the spec's
    fill); masked=True applies the where(mask, e, -1e9) select generally."""
    nc = bacc.Bacc()
    P = nc.declare_dram_parameter
    encT = P("encT", [BL, E, S], BF16, False)   # enc' = enc + cov x u, transposed
    encN = P("encN", [BL, S, E], BF16, False)   # original enc, natural layout
    wenc = P("wenc", [E, H], BF16, False)
    wdeca = P("wdeca", [KD, H], BF16, False)       # rows 0..511 W_dec, 512 bias, rest 0
    dechTa = P("dechTa", [KD, BL], BF16, False)    # rows 0..511 dec_h.T, 512 ones, rest 0
    vcol = P("vcol", [H], BF16, False)
    covf = P("covf", [BL, S], F32, False)
    if masked:
        emul = P("emul", [BL, S], F32, False)
        eadd = P("eadd", [BL, S], F32, False)
    o_a = P("o_a", [BL, S], F32, True)
    o_h = P("o_h", [BL, E], F32, True)
    o_c = P("o_c", [BL, S], F32, True)
    o_l = P("o_l", [BL, 1], F32, True)

    with tile.TileContext(nc) as tc:
        with (
            tc.tile_pool(name="wp", bufs=1) as wp,
            tc.tile_pool(name="bigp", bufs=1) as bigp,
            tc.tile_pool(name="workp", bufs=3) as workp,
            tc.tile_pool(name="rowp", bufs=2) as rowp,
            tc.tile_pool(name="psum", bufs=1, space="PSUM") as pp,
            tc.tile_pool(name="dramp", bufs=2, space="DRAM") as dp,
        ):
            # ---- PE warm-up: keep the HAM activity window busy while the
            # weight/encoder DMAs land, so the first real matmuls run at 2.4GHz.
            warm = wp.tile([128, 512], BF16)
            nc.gpsimd.memset(warm[:], 0.0)
            ident1 = wp.tile([1, 1], BF16)
            nc.gpsimd.memset(ident1[:], 1.0)
            for _ in range(14):
                wps = pp.tile([128, 512], F32, tag="prj", bufs=4)
                nc.tensor.matmul(wps[:], warm[:, 0:128], warm[:], start=True, stop=True)

            # ---- persistent weights (small ones first; wenc split per chunk) ----
            wdec_sb = wp.tile([128, 5, H], BF16)
            nc.sync.dma_start(wdec_sb[:], wdeca[:].rearrange("(c p) h -> p c h", p=128))
            dech_sb = wp.tile([128, 5, BL], BF16)
            nc.sync.dma_start(dech_sb[:], dechTa[:].rearrange("(c p) b -> p c b", p=128))
            v_sb = wp.tile([128, 4], BF16)
            nc.sync.dma_start(v_sb[:], vcol[:].rearrange("(c p) -> p c", p=128))
            wenc_view = wenc[:].rearrange("(c p) h -> c p h", p=128)
            wenc_cs = []
            et0 = []
            for e in range(8):
                wc = wp.tile([128, H], BF16, name=f"wenc_{e}")
                nc.sync.dma_start(wc[:], wenc_view[e])
                wenc_cs.append(wc)
                et = bigp.tile([128, S], BF16, tag="encT", bufs=32)
                nc.sync.dma_start(et[:], encT[0, e * 128:(e + 1) * 128, :])
                et0.append(et)

            # ---- transposed dec projection: dec_colT[p, c, b] = (dec_h[b] @ W_dec + bias)[c*128+p]
            dec_colT = wp.tile([128, 4, BL], F32)
            for c in range(4):
                dtp = pp.tile([128, BL], F32, tag="eps", bufs=2)
                for k in range(5):
                    nc.tensor.matmul(
                        dtp[:], wdec_sb[:, k, c * 128:(c + 1) * 128], dech_sb[:, k, :],
                        start=(k == 0), stop=(k == 4),
                    )
                nc.vector.tensor_copy(dec_colT[:, c, :], dtp[:])

            # ---- emission helpers (software-pipelined schedule) ----
            def emit_proj_block(b, sh, ets):
                ths = []
                for h in range(4):
                    prj = pp.tile([128, 512], F32, tag="prj", bufs=4)
                    for e in range(8):
                        nc.tensor.matmul(
                            prj[:],
                            wenc_cs[e][:, h * 128:(h + 1) * 128],
                            ets[e][:, sh * 512:(sh + 1) * 512],
                            start=(e == 0), stop=(e == 7),
                        )
                    th = workp.tile([128, 512], BF16, tag="tanh", bufs=10)
                    nc.scalar.activation(th[:], prj[:], AF.Tanh,
                                         bias=dec_colT[:, h, b:b + 1])
                    ths.append(th)
                return ths

            def emit_escore(b, sh, ths, eps_out):
                for h in range(4):
                    nc.tensor.matmul(
                        eps_out[:], v_sb[:, h:h + 1], ths[h][:],
                        start=(h == 0), stop=(h == 3),
                    )

            def emit_softmax_masked(b, eps0, eps1):
                """General path: where(mask, e, -1e9), max-subtracted softmax."""
                p_row = rowp.tile([1, S], BF16, tag="p_row")
                emul_r = rowp.tile([1, S], F32, tag="emul_r")
                nc.sync.dma_start(emul_r[:], emul[b:b + 1, :])
                eadd_r = rowp.tile([1, S], F32, tag="eadd_r")
                nc.sync.dma_start(eadd_r[:], eadd[b:b + 1, :])
                em = rowp.tile([1, S], F32, tag="em")
                nc.vector.tensor_copy(em[:, 0:512], eps0[:])
                nc.vector.tensor_copy(em[:, 512:1024], eps1[:])
                nc.vector.tensor_tensor(em[:], em[:], emul_r[:], op=ALU.mult)
                nc.vector.tensor_tensor(em[:], em[:], eadd_r[:], op=ALU.add)
                mx = rowp.tile([1, 1], F32, tag="mx")
                nc.vector.reduce_max(mx[:], em[:], axis=AX.X)
                nc.vector.tensor_scalar_mul(mx[:], mx[:], -1.0)
                sm = rowp.tile([1, 1], F32, tag="sm")
                nc.scalar.activation(p_row[:], em[:], AF.Exp, bias=mx[:, 0:1],
                                     accum_out=sm[:])
                rv = rowp.tile([1, 1], F32, tag="rv")
                nc.vector.reciprocal(rv[:], sm[:])
                scr = dp.tile([S], BF16, tag="scr")
                nc.sync.dma_start(scr[None, :], p_row[:])
                aT = workp.tile([128, BL], BF16, tag="aT")
                nc.sync.dma_start(aT[:], scr[:].rearrange("(c p) -> p c", p=128))
                return aT, rv, p_row

            def emit_exp_half(b, sh, eps, p_row, dma=None, accum_out=None):
                """exp of one e-score half straight from PSUM + column bounce."""
                dma = dma or nc.sync
                nc.scalar.activation(p_row[:, sh * 512:(sh + 1) * 512], eps[:], AF.Exp,
                                     accum_out=accum_out)
                scr = dp.tile([512], BF16, tag="scrh", bufs=3)
                dma.dma_start(scr[None, :], p_row[:, sh * 512:(sh + 1) * 512])
                aTh = workp.tile([128, 4], BF16, tag="aTh", bufs=4)
                dma.dma_start(aTh[:], scr[:].rearrange("(c p) -> p c", p=128))
                return aTh

            def emit_exp_half_pe(b, sh, eps, p_row, accum_out):
                """exp + PE-transpose to columns (no DRAM bounce; for the
                kernel tail where the bounce latency would be exposed)."""
                nc.scalar.activation(p_row[:, sh * 512:(sh + 1) * 512], eps[:],
                                     AF.Exp, accum_out=accum_out)
                atp = pp.tile([128, 8], BF16, tag="eps", bufs=2)
                for j in range(4):
                    s0 = (sh * 4 + j) * 128
                    nc.tensor.transpose(atp[:, 2 * j:2 * j + 1],
                                        p_row[:, s0:s0 + 128], ident1[:])
                aTh = workp.tile([128, 4], BF16, tag="aTh", bufs=4)
                for j in range(4):
                    nc.vector.tensor_copy(aTh[:, j:j + 1], atp[:, 2 * j:2 * j + 1])
                return aTh

            def emit_hstar_half(b, sh, aTh, hp0, hp1, ens=None):
                for j in range(4):
                    sc = sh * 4 + j
                    if ens is not None:
                        en = ens[sc]
                    else:
                        en = workp.tile([128, E], BF16, tag="encN", bufs=10)
                        nc.sync.dma_start(en[:], encN[b, sc * 128:(sc + 1) * 128, :])
                    first = (sh == 0 and j == 0)
                    last = (sh == 1 and j == 3)
                    nc.tensor.matmul(hp0[:], aTh[:, j:j + 1], en[:, 0:512],
                                     start=first, stop=last)
                    nc.tensor.matmul(hp1[:], aTh[:, j:j + 1], en[:, 512:1024],
                                     start=first, stop=last)

            def finish_hstar(b, hp0, hp1, rv):
                # scale on ACT so the tail normalization runs in parallel with
                # the DVE output chain
                hs = rowp.tile([1, E], F32, tag="hs")
                nc.scalar.mul(hs[:, 0:512], hp0[:], rv[:, 0:1])
                nc.scalar.mul(hs[:, 512:1024], hp1[:], rv[:, 0:1])
                nc.sync.dma_start(o_h[b:b + 1, :], hs[:])

            def emit_hstar_full(b, aT, rv):
                hp0 = pp.tile([1, 512], F32, tag="acc", bufs=2)
                hp1 = pp.tile([1, 512], F32, tag="acc", bufs=2)
                for sc in range(8):
                    en = workp.tile([128, E], BF16, tag="encN", bufs=10)
                    nc.sync.dma_start(en[:], encN[b, sc * 128:(sc + 1) * 128, :])
                    nc.tensor.matmul(hp0[:], aT[:, sc:sc + 1], en[:, 0:512],
                                     start=(sc == 0), stop=(sc == 7))
                    nc.tensor.matmul(hp1[:], aT[:, sc:sc + 1], en[:, 512:1024],
                                     start=(sc == 0), stop=(sc == 7))
                finish_hstar(b, hp0, hp1, rv)

            def emit_outputs(b, p_row, rv):
                """a, new_coverage, covloss — off the PE critical path."""
                covf_r = rowp.tile([1, S], F32, tag="covf_r")
                nc.sync.dma_start(covf_r[:], covf[b:b + 1, :])
                ar = rowp.tile([1, S], F32, tag="ar")
                nc.scalar.mul(ar[:], p_row[:], rv[:, 0:1])
                nc.sync.dma_start(o_a[b:b + 1, :], ar[:])
                # new_coverage = cov + a via DMA-accumulate onto the
                # zero-initialized output (same gpsimd queue -> serialized RMW)
                nc.gpsimd.dma_start(o_c[b:b + 1, :], covf_r[:])
                nc.gpsimd.dma_start(o_c[b:b + 1, :], ar[:], accum_op=ALU.add)
                mnr = rowp.tile([1, S], F32, tag="mnr")
                nc.vector.tensor_tensor(mnr[:], ar[:], covf_r[:], op=ALU.min)
                cl = rowp.tile([1, 1], F32, tag="cl")
                nc.vector.reduce_sum(cl[:], mnr[:], axis=AX.X)
                nc.sync.dma_start(o_l[b:b + 1, :], cl[:])

            def load_encT(b):
                ets = []
                for e in range(8):
                    et = bigp.tile([128, S], BF16, tag="encT", bufs=32)
                    nc.sync.dma_start(et[:], encT[b, e * 128:(e + 1) * 128, :])
                    ets.append(et)
                return ets

            # ---- main per-batch pipeline ----
            def emit_softmax_fast(b, eps0, eps1):
                p_row = rowp.tile([1, S], BF16, tag="p_row")
                sm0 = rowp.tile([1, 1], F32, tag="sm0")
                sm1 = rowp.tile([1, 1], F32, tag="sm1")
                nc.scalar.activation(p_row[:, 0:512], eps0[:], AF.Exp,
                                     accum_out=sm0[:])
                nc.scalar.activation(p_row[:, 512:1024], eps1[:], AF.Exp,
                                     accum_out=sm1[:])
                sm = rowp.tile([1, 1], F32, tag="sm")
                nc.vector.tensor_tensor(sm[:], sm0[:], sm1[:], op=ALU.add)
                rv = rowp.tile([1, 1], F32, tag="rv")
                nc.vector.reciprocal(rv[:], sm[:])
                scr = dp.tile([S], BF16, tag="scr")
                nc.sync.dma_start(scr[None, :], p_row[:])
                aT = workp.tile([128, BL], BF16, tag="aT")
                nc.sync.dma_start(aT[:], scr[:].rearrange("(c p) -> p c", p=128))
                return aT, rv, p_row

            pending = None
            ets = et0
            for b in range(BL):
                last = (b == BL - 1) and not masked
                eps0 = pp.tile([1, 512], F32, tag="eps", bufs=2)
                eps1 = pp.tile([1, 512], F32, tag="eps", bufs=2)
                ths0 = emit_proj_block(b, 0, ets)
                if pending is not None:
                    emit_hstar_full(*pending)
                    pending = None
                if b + 1 < BL:
                    ets_next = load_encT(b + 1)
                if last:
                    # Pull the sh0 half of the softmax/h_star chain forward so
                    # only the sh1 half-chain remains serial at the kernel tail.
                    # encN tiles are preloaded so the tail bounce DMAs see an
                    # empty queue; the bounces issue from ScalarE (no tanh
                    # stream left to block there).
                    ens = []
                    for sc in range(8):
                        en = workp.tile([128, E], BF16, tag="encN", bufs=10)
                        nc.sync.dma_start(en[:], encN[b, sc * 128:(sc + 1) * 128, :])
                        ens.append(en)
                    emit_escore(b, 0, ths0, eps0)
                    p_row = rowp.tile([1, S], BF16, tag="p_row")
                    sm0 = rowp.tile([1, 1], F32, tag="sm0")
                    aT0 = emit_exp_half_pe(b, 0, eps0, p_row, sm0[:])
                    ths1 = emit_proj_block(b, 1, ets)
                    hp0 = pp.tile([1, 512], F32, tag="acc", bufs=2)
                    hp1 = pp.tile([1, 512], F32, tag="acc", bufs=2)
                    emit_hstar_half(b, 0, aT0, hp0, hp1, ens=ens)
                    emit_escore(b, 1, ths1, eps1)
                    sm1 = rowp.tile([1, 1], F32, tag="sm1")
                    aT1 = emit_exp_half_pe(b, 1, eps1, p_row, sm1[:])
                    sm = rowp.tile([1, 1], F32, tag="sm")
                    nc.vector.tensor_tensor(sm[:], sm0[:], sm1[:], op=ALU.add)
                    rv = rowp.tile([1, 1], F32, tag="rv")
                    nc.vector.reciprocal(rv[:], sm[:])
                    emit_outputs(b, p_row, rv)
                    emit_hstar_half(b, 1, aT1, hp0, hp1, ens=ens)
                    finish_hstar(b, hp0, hp1, rv)
                    continue
                ths1 = emit_proj_block(b, 1, ets)
                emit_escore(b, 0, ths0, eps0)
                emit_escore(b, 1, ths1, eps1)
                if masked:
                    aT, rv, p_row = emit_softmax_masked(b, eps0, eps1)
                else:
                    aT, rv, p_row = emit_softmax_fast(b, eps0, eps1)
                emit_outputs(b, p_row, rv)
                pending = (b, aT, rv)
                if b + 1 < BL:
                    ets = ets_next
            if pending is not None:
                emit_hstar_full(*pending)

    return nc


def get_graph(masked):
    global _graphs
    if _graphs.get(masked) is None:
        g = _build_graph(masked)
        if not g.is_finalized():
            g.finalize()
        _graphs[masked] = g
    return _graphs[masked]


def kernel(enc_h, dec_h, coverage_vec, mask, W_enc, W_dec, w_cov, bias, v):
    bf = ml_dtypes.bfloat16
    enc_h = np.asarray(enc_h)
    dec_h = np.asarray(dec_h, dtype=np.float32)
    coverage_vec = np.asarray(coverage_vec, dtype=np.float32)
    mask_b = np.asarray(mask).astype(bool)
    W_enc = np.asarray(W_enc, dtype=np.float32)
    W_dec = np.asarray(W_dec, dtype=np.float32)
    w_cov = np.asarray(w_cov, dtype=np.float32)
    bias = np.asarray(bias, dtype=np.float32)
    v = np.asarray(v, dtype=np.float32)

    # Fast path: all-ones mask (the spec's fill) and scores that cannot
    # overflow exp without max-subtraction.
    masked = (not bool(mask_b.all())) or float(np.abs(v).sum()) > 80.0

    # Fold the coverage outer-product cov[s]*w_cov[h] into the main
    # contraction: find least-norm u with W_enc.T @ u = w_cov, then
    # enc' = enc + cov x u satisfies enc' @ W_enc = enc @ W_enc + cov x w_cov.
    W64 = W_enc.astype(np.float64)
    u = (W64 @ np.linalg.solve(W64.T @ W64, w_cov.astype(np.float64))).astype(np.float32)

    wdeca = np.zeros((KD, H), np.float32)
    wdeca[:H] = W_dec
    wdeca[H] = bias[0]
    wdeca = wdeca.astype(bf)
    wenc_bf = W_enc.astype(bf)
    vcol = v.astype(bf)

    mask_f = mask_b.astype(np.float32)
    in_maps = []
    for c in range(NCORES):
        sl = slice(c * BL, (c + 1) * BL)
        enc_c = np.asarray(enc_h[sl], dtype=np.float32)
        cov_c = np.ascontiguousarray(coverage_vec[sl])
        encN_c = enc_c.astype(bf)
        encT_c = np.ascontiguousarray(
            (enc_c + cov_c[:, :, None] * u[None, None, :]).transpose(0, 2, 1)
        ).astype(bf)
        dechTa = np.zeros((KD, BL), np.float32)
        dechTa[:H] = dec_h[sl].T
        dechTa[H] = 1.0
        m = {
            "encT": encT_c,
            "encN": encN_c,
            "wenc": wenc_bf,
            "wdeca": wdeca,
            "dechTa": dechTa.astype(bf),
            "vcol": vcol,
            "covf": cov_c,
        }
        if masked:
            m_c = np.ascontiguousarray(mask_f[sl])
            m["emul"] = m_c
            m["eadd"] = ((m_c - 1.0) * 1e9).astype(np.float32)
        in_maps.append(m)

    run = run_bass_kernel_spmd(get_graph(masked), in_maps, list(range(NCORES)))
    global LAST_RUN
    LAST_RUN = run
    res = run.results

    a = np.concatenate([np.asarray(r["o_a"], np.float32) for r in res], axis=0)
    h_star = np.concatenate([np.asarray(r["o_h"], np.float32) for r in res], axis=0)
    new_cov = np.concatenate([np.asarray(r["o_c"], np.float32) for r in res], axis=0)
    covloss = np.concatenate(
        [np.asarray(r["o_l"], np.float32).reshape(BL) for r in res], axis=0
    )
    return (a, h_star, new_cov, covloss)


# revision 50
# speedup vs baseline: 1.1659x; 1.1659x over previous
"""Trainium2 Bass kernel for coverage-attention (Bahdanau-style with coverage).

Reference computation (per batch row b):
  proj_sum[s,h] = enc_h[b,s,:] @ W_enc[:,h] + (dec_h[b] @ W_dec)[h] + cov[b,s]*w_cov[h]
  e[s]     = sum_h tanh(proj_sum[s,h] + bias) * v[h]
  a        = softmax(where(mask, e, -1e9))
  covloss  = sum_s min(a, cov)
  h_star   = sum_s a[s] * enc_h[b,s,:]
  new_cov  = cov + a

Strategy: data-parallel over batch (64 -> 8 per NeuronCore). Compute in bf16
on the TensorEngine (PSUM accumulates f32). The host pre-shards and
pre-marshals layouts: enc_h is shipped both natural [b,s,e] (for h_star) and
transposed [b,e,s] (for the projection contraction over e), already cast to
bf16. The coverage outer-product cov[s]*w_cov[h] is folded into the
transposed copy on the host (enc' = enc + cov x u with W_enc.T u = w_cov),
and the dec-projection + bias enters as the tanh's per-partition bias, so
the device-side projection is a clean K=128 matmul chain. proj is computed
transposed ([h_part, s_free]) so the v-weighted tanh reduction is a TensorE
matmul. The per-batch emission is software-pipelined: batch b-1's h_star
matmuls hide inside batch b's projection stream, softmax runs unnormalized
(exp straight from the e-score PSUM, normalization folded into the h_star
epilogue), and the attention row is transposed to matmul-column layout via a
small DRAM bounce. A short warm-up matmul burst holds the PE's HAM activity
window open while the first DMAs land.
"""

import sys

sys.path.insert(0, "/opt/trn_rl_repo")

import numpy as np
import ml_dtypes

import concourse.bass as bass
import concourse.mybir as mybir
import concourse.tile as tile
from concourse import bacc
from concourse.bass_utils import run_bass_kernel_spmd

B, S, E, H = 64, 1024, 1024, 512  # E = 2H
NCORES = 8
BL = B // NCORES  # 8 local batch rows per core
KD = 640  # padded contraction dim for dec proj (512 + 1 bias row, padded to 5*128)

BF16 = mybir.dt.bfloat16
F32 = mybir.dt.float32
AF = mybir.ActivationFunctionType
ALU = mybir.AluOpType
AX = mybir.AxisListType

_graphs = {}
LAST_RUN = None  # BassKernelResults of the most recent run (exec_time_ns when traced)


def _build_graph(masked):
    """masked=False is the fast path used when mask is all-ones (the spec's
    fill); masked=True applies the where(mask, e, -1e9) select generally."""
    nc = bacc.Bacc()
    P = nc.declare_dram_parameter
    encT = P("encT", [BL, E, S], BF16, False)   # enc' = enc + cov x u, transposed
    encN = P("encN", [BL, S, E], BF16, False)   # original enc, natural layout
    wenc = P("wenc", [E, H], BF16, False)
    wdeca = P("wdeca", [KD, H], BF16, False)       # rows 0..511 W_dec, 512 bias, rest 0
    dechTa = P("dechTa", [KD, BL], BF16, False)    # rows 0..511 dec_h.T, 512 ones, rest 0
    vcol = P("vcol", [H], BF16, False)
    covf = P("covf", [BL, S], F32, False)
    if masked:
        emul = P("emul", [BL, S], F32, False)
        eadd = P("eadd", [BL, S], F32, False)
    o_a = P("o_a", [BL, S], F32, True)
    o_h = P("o_h", [BL, E], F32, True)
    o_c = P("o_c", [BL, S], F32, True)
    o_l = P("o_l", [BL, 1], F32, True)

    with tile.TileContext(nc) as tc:
        with (
            tc.tile_pool(name="wp", bufs=1) as wp,
            tc.tile_pool(name="bigp", bufs=1) as bigp,
            tc.tile_pool(name="workp", bufs=3) as workp,
            tc.tile_pool(name="rowp", bufs=2) as rowp,
            tc.tile_pool(name="psum", bufs=1, space="PSUM") as pp,
            tc.tile_pool(name="dramp", bufs=2, space="DRAM") as dp,
        ):
            # ---- PE warm-up: keep the HAM activity window busy while the
            # weight/encoder DMAs land, so the first real matmuls run at 2.4GHz.
            warm = wp.tile([128, 512], BF16)
            nc.gpsimd.memset(warm[:], 0.0)
            ident1 = wp.tile([1, 1], BF16)
            nc.gpsimd.memset(ident1[:], 1.0)
            for _ in range(12):
                wps = pp.tile([128, 512], F32, tag="prj", bufs=4)
                nc.tensor.matmul(wps[:], warm[:, 0:128], warm[:], start=True, stop=True)

            # ---- persistent weights (small ones first; wenc split per chunk) ----
            wdec_sb = wp.tile([128, 5, H], BF16)
            nc.sync.dma_start(wdec_sb[:], wdeca[:].rearrange("(c p) h -> p c h", p=128))
            dech_sb = wp.tile([128, 5, BL], BF16)
            nc.sync.dma_start(dech_sb[:], dechTa[:].rearrange("(c p) b -> p c b", p=128))
            v_sb = wp.tile([128, 4], BF16)
            nc.sync.dma_start(v_sb[:], vcol[:].rearrange("(c p) -> p c", p=128))
            wenc_view = wenc[:].rearrange("(c p) h -> c p h", p=128)
            wenc_cs = []
            et0 = []
            for e in range(8):
                wc = wp.tile([128, H], BF16, name=f"wenc_{e}")
                nc.sync.dma_start(wc[:], wenc_view[e])
                wenc_cs.append(wc)
                et = bigp.tile([128, S], BF16, tag="encT", bufs=32)
                nc.sync.dma_start(et[:], encT[0, e * 128:(e + 1) * 128, :])
                et0.append(et)

            # ---- transposed dec projection: dec_colT[p, c, b] = (dec_h[b] @ W_dec + bias)[c*128+p]
            dec_colT = wp.tile([128, 4, BL], F32)
            for c in range(4):
                dtp = pp.tile([128, BL], F32, tag="eps", bufs=2)
                for k in range(5):
                    nc.tensor.matmul(
                        dtp[:], wdec_sb[:, k, c * 128:(c + 1) * 128], dech_sb[:, k, :],
                        start=(k == 0), stop=(k == 4),
                    )
                nc.vector.tensor_copy(dec_colT[:, c, :], dtp[:])

            # ---- emission helpers (software-pipelined schedule) ----
            def emit_proj_block(b, sh, ets):
                ths = []
                for h in range(4):
                    prj = pp.tile([128, 512], F32, tag="prj", bufs=4)
                    for e in range(8):
                        nc.tensor.matmul(
                            prj[:],
                            wenc_cs[e][:, h * 128:(h + 1) * 128],
                            ets[e][:, sh * 512:(sh + 1) * 512],
                            start=(e == 0), stop=(e == 7),
                        )
                    th = workp.tile([128, 512], BF16, tag="tanh", bufs=10)
                    nc.scalar.activation(th[:], prj[:], AF.Tanh,
                                         bias=dec_colT[:, h, b:b + 1])
                    ths.append(th)
                return ths

            def emit_escore(b, sh, ths, eps_out):
                for h in range(4):
                    nc.tensor.matmul(
                        eps_out[:], v_sb[:, h:h + 1], ths[h][:],
                        start=(h == 0), stop=(h == 3),
                    )

            def emit_softmax_masked(b, eps0, eps1):
                """General path: where(mask, e, -1e9), max-subtracted softmax."""
                p_row = rowp.tile([1, S], BF16, tag="p_row")
                emul_r = rowp.tile([1, S], F32, tag="emul_r")
                nc.sync.dma_start(emul_r[:], emul[b:b + 1, :])
                eadd_r = rowp.tile([1, S], F32, tag="eadd_r")
                nc.sync.dma_start(eadd_r[:], eadd[b:b + 1, :])
                em = rowp.tile([1, S], F32, tag="em")
                nc.vector.tensor_copy(em[:, 0:512], eps0[:])
                nc.vector.tensor_copy(em[:, 512:1024], eps1[:])
                nc.vector.tensor_tensor(em[:], em[:], emul_r[:], op=ALU.mult)
                nc.vector.tensor_tensor(em[:], em[:], eadd_r[:], op=ALU.add)
                mx = rowp.tile([1, 1], F32, tag="mx")
                nc.vector.reduce_max(mx[:], em[:], axis=AX.X)
                nc.vector.tensor_scalar_mul(mx[:], mx[:], -1.0)
                sm = rowp.tile([1, 1], F32, tag="sm")
                nc.scalar.activation(p_row[:], em[:], AF.Exp, bias=mx[:, 0:1],
                                     accum_out=sm[:])
                rv = rowp.tile([1, 1], F32, tag="rv")
                nc.vector.reciprocal(rv[:], sm[:])
                scr = dp.tile([S], BF16, tag="scr")
                nc.sync.dma_start(scr[None, :], p_row[:])
                aT = workp.tile([128, BL], BF16, tag="aT")
                nc.sync.dma_start(aT[:], scr[:].rearrange("(c p) -> p c", p=128))
                return aT, rv, p_row

            def emit_exp_half(b, sh, eps, p_row, dma=None, accum_out=None):
                """exp of one e-score half straight from PSUM + column bounce."""
                dma = dma or nc.sync
                nc.scalar.activation(p_row[:, sh * 512:(sh + 1) * 512], eps[:], AF.Exp,
                                     accum_out=accum_out)
                scr = dp.tile([512], BF16, tag="scrh", bufs=3)
                dma.dma_start(scr[None, :], p_row[:, sh * 512:(sh + 1) * 512])
                aTh = workp.tile([128, 4], BF16, tag="aTh", bufs=4)
                dma.dma_start(aTh[:], scr[:].rearrange("(c p) -> p c", p=128))
                return aTh

            def emit_exp_half_pe(b, sh, eps, p_row, accum_out):
                """exp + PE-transpose to columns (no DRAM bounce; for the
                kernel tail where the bounce latency would be exposed)."""
                nc.scalar.activation(p_row[:, sh * 512:(sh + 1) * 512], eps[:],
                                     AF.Exp, accum_out=accum_out)
                atp = pp.tile([128, 8], BF16, tag="eps", bufs=2)
                for j in range(4):
                    s0 = (sh * 4 + j) * 128
                    nc.tensor.transpose(atp[:, 2 * j:2 * j + 1],
                                        p_row[:, s0:s0 + 128], ident1[:])
                aTh = workp.tile([128, 4], BF16, tag="aTh", bufs=4)
                for j in range(4):
                    nc.vector.tensor_copy(aTh[:, j:j + 1], atp[:, 2 * j:2 * j + 1])
                return aTh

            def emit_hstar_half(b, sh, aTh, hp0, hp1, ens=None):
                for j in range(4):
                    sc = sh * 4 + j
                    if ens is not None:
                        en = ens[sc]
                    else:
                        en = workp.tile([128, E], BF16, tag="encN", bufs=10)
                        nc.sync.dma_start(en[:], encN[b, sc * 128:(sc + 1) * 128, :])
                    first = (sh == 0 and j == 0)
                    last = (sh == 1 and j == 3)
                    nc.tensor.matmul(hp0[:], aTh[:, j:j + 1], en[:, 0:512],
                                     start=first, stop=last)
                    nc.tensor.matmul(hp1[:], aTh[:, j:j + 1], en[:, 512:1024],
                                     start=first, stop=last)

            def finish_hstar(b, hp0, hp1, rv):
                # scale on ACT so the tail normalization runs in parallel with
                # the DVE output chain
                hs = rowp.tile([1, E], F32, tag="hs")
                nc.scalar.mul(hs[:, 0:512], hp0[:], rv[:, 0:1])
                nc.scalar.mul(hs[:, 512:1024], hp1[:], rv[:, 0:1])
                nc.sync.dma_start(o_h[b:b + 1, :], hs[:])

            def emit_hstar_full(b, aT, rv):
                hp0 = pp.tile([1, 512], F32, tag="acc", bufs=2)
                hp1 = pp.tile([1, 512], F32, tag="acc", bufs=2)
                for sc in range(8):
                    en = workp.tile([128, E], BF16, tag="encN", bufs=10)
                    nc.sync.dma_start(en[:], encN[b, sc * 128:(sc + 1) * 128, :])
                    nc.tensor.matmul(hp0[:], aT[:, sc:sc + 1], en[:, 0:512],
                                     start=(sc == 0), stop=(sc == 7))
                    nc.tensor.matmul(hp1[:], aT[:, sc:sc + 1], en[:, 512:1024],
                                     start=(sc == 0), stop=(sc == 7))
                finish_hstar(b, hp0, hp1, rv)

            def emit_outputs(b, p_row, rv):
                """a, new_coverage, covloss — off the PE critical path."""
                covf_r = rowp.tile([1, S], F32, tag="covf_r")
                nc.sync.dma_start(covf_r[:], covf[b:b + 1, :])
                ar = rowp.tile([1, S], F32, tag="ar")
                nc.scalar.mul(ar[:], p_row[:], rv[:, 0:1])
                nc.sync.dma_start(o_a[b:b + 1, :], ar[:])
                # new_coverage = cov + a via DMA-accumulate onto the
                # zero-initialized output (same gpsimd queue -> serialized RMW)
                nc.gpsimd.dma_start(o_c[b:b + 1, :], covf_r[:])
                nc.gpsimd.dma_start(o_c[b:b + 1, :], ar[:], accum_op=ALU.add)
                mnr = rowp.tile([1, S], F32, tag="mnr")
                nc.vector.tensor_tensor(mnr[:], ar[:], covf_r[:], op=ALU.min)
                cl = rowp.tile([1, 1], F32, tag="cl")
                nc.vector.reduce_sum(cl[:], mnr[:], axis=AX.X)
                nc.sync.dma_start(o_l[b:b + 1, :], cl[:])

            def load_encT(b):
                ets = []
                for e in range(8):
                    et = bigp.tile([128, S], BF16, tag="encT", bufs=32)
                    nc.sync.dma_start(et[:], encT[b, e * 128:(e + 1) * 128, :])
                    ets.append(et)
                return ets

            # ---- main per-batch pipeline ----
            def emit_softmax_fast(b, eps0, eps1):
                p_row = rowp.tile([1, S], BF16, tag="p_row")
                sm0 = rowp.tile([1, 1], F32, tag="sm0")
                sm1 = rowp.tile([1, 1], F32, tag="sm1")
                nc.scalar.activation(p_row[:, 0:512], eps0[:], AF.Exp,
                                     accum_out=sm0[:])
                nc.scalar.activation(p_row[:, 512:1024], eps1[:], AF.Exp,
                                     accum_out=sm1[:])
                sm = rowp.tile([1, 1], F32, tag="sm")
                nc.vector.tensor_tensor(sm[:], sm0[:], sm1[:], op=ALU.add)
                rv = rowp.tile([1, 1], F32, tag="rv")
                nc.vector.reciprocal(rv[:], sm[:])
                scr = dp.tile([S], BF16, tag="scr")
                nc.sync.dma_start(scr[None, :], p_row[:])
                aT = workp.tile([128, BL], BF16, tag="aT")
                nc.sync.dma_start(aT[:], scr[:].rearrange("(c p) -> p c", p=128))
                return aT, rv, p_row

            pending = None
            ets = et0
            for b in range(BL):
                last = (b == BL - 1) and not masked
                eps0 = pp.tile([1, 512], F32, tag="eps", bufs=2)
                eps1 = pp.tile([1, 512], F32, tag="eps", bufs=2)
                ths0 = emit_proj_block(b, 0, ets)
                if pending is not None:
                    emit_hstar_full(*pending)
                    pending = None
                if b + 1 < BL:
                    ets_next = load_encT(b + 1)
                if last:
                    # Pull the sh0 half of the softmax/h_star chain forward so
                    # only the sh1 half-chain remains serial at the kernel tail.
                    # encN tiles are preloaded so the tail bounce DMAs see an
                    # empty queue; the bounces issue from ScalarE (no tanh
                    # stream left to block there).
                    ens = []
                    for sc in range(8):
                        en = workp.tile([128, E], BF16, tag="encN", bufs=10)
                        nc.sync.dma_start(en[:], encN[b, sc * 128:(sc + 1) * 128, :])
                        ens.append(en)
                    emit_escore(b, 0, ths0, eps0)
                    p_row = rowp.tile([1, S], BF16, tag="p_row")
                    sm0 = rowp.tile([1, 1], F32, tag="sm0")
                    aT0 = emit_exp_half_pe(b, 0, eps0, p_row, sm0[:])
                    ths1 = emit_proj_block(b, 1, ets)
                    hp0 = pp.tile([1, 512], F32, tag="acc", bufs=2)
                    hp1 = pp.tile([1, 512], F32, tag="acc", bufs=2)
                    emit_hstar_half(b, 0, aT0, hp0, hp1, ens=ens)
                    emit_escore(b, 1, ths1, eps1)
                    sm1 = rowp.tile([1, 1], F32, tag="sm1")
                    aT1 = emit_exp_half_pe(b, 1, eps1, p_row, sm1[:])
                    sm = rowp.tile([1, 1], F32, tag="sm")
                    nc.vector.tensor_tensor(sm[:], sm0[:], sm1[:], op=ALU.add)
                    rv = rowp.tile([1, 1], F32, tag="rv")
                    nc.vector.reciprocal(rv[:], sm[:])
                    emit_outputs(b, p_row, rv)
                    emit_hstar_half(b, 1, aT1, hp0, hp1, ens=ens)
                    finish_hstar(b, hp0, hp1, rv)
                    continue
                ths1 = emit_proj_block(b, 1, ets)
                emit_escore(b, 0, ths0, eps0)
                emit_escore(b, 1, ths1, eps1)
                if masked:
                    aT, rv, p_row = emit_softmax_masked(b, eps0, eps1)
                else:
                    aT, rv, p_row = emit_softmax_fast(b, eps0, eps1)
                emit_outputs(b, p_row, rv)
                pending = (b, aT, rv)
                if b + 1 < BL:
                    ets = ets_next
            if pending is not None:
                emit_hstar_full(*pending)

    return nc


def get_graph(masked):
    global _graphs
    if _graphs.get(masked) is None:
        g = _build_graph(masked)
        if not g.is_finalized():
            g.finalize()
        _graphs[masked] = g
    return _graphs[masked]


def kernel(enc_h, dec_h, coverage_vec, mask, W_enc, W_dec, w_cov, bias, v):
    bf = ml_dtypes.bfloat16
    enc_h = np.asarray(enc_h)
    dec_h = np.asarray(dec_h, dtype=np.float32)
    coverage_vec = np.asarray(coverage_vec, dtype=np.float32)
    mask_b = np.asarray(mask).astype(bool)
    W_enc = np.asarray(W_enc, dtype=np.float32)
    W_dec = np.asarray(W_dec, dtype=np.float32)
    w_cov = np.asarray(w_cov, dtype=np.float32)
    bias = np.asarray(bias, dtype=np.float32)
    v = np.asarray(v, dtype=np.float32)

    # Fast path: all-ones mask (the spec's fill) and scores that cannot
    # overflow exp without max-subtraction.
    masked = (not bool(mask_b.all())) or float(np.abs(v).sum()) > 80.0

    # Fold the coverage outer-product cov[s]*w_cov[h] into the main
    # contraction: find least-norm u with W_enc.T @ u = w_cov, then
    # enc' = enc + cov x u satisfies enc' @ W_enc = enc @ W_enc + cov x w_cov.
    W64 = W_enc.astype(np.float64)
    u = (W64 @ np.linalg.solve(W64.T @ W64, w_cov.astype(np.float64))).astype(np.float32)

    wdeca = np.zeros((KD, H), np.float32)
    wdeca[:H] = W_dec
    wdeca[H] = bias[0]
    wdeca = wdeca.astype(bf)
    wenc_bf = W_enc.astype(bf)
    vcol = v.astype(bf)

    mask_f = mask_b.astype(np.float32)
    in_maps = []
    for c in range(NCORES):
        sl = slice(c * BL, (c + 1) * BL)
        enc_c = np.asarray(enc_h[sl], dtype=np.float32)
        cov_c = np.ascontiguousarray(coverage_vec[sl])
        encN_c = enc_c.astype(bf)
        encT_c = np.ascontiguousarray(
            (enc_c + cov_c[:, :, None] * u[None, None, :]).transpose(0, 2, 1)
        ).astype(bf)
        dechTa = np.zeros((KD, BL), np.float32)
        dechTa[:H] = dec_h[sl].T
        dechTa[H] = 1.0
        m = {
            "encT": encT_c,
            "encN": encN_c,
            "wenc": wenc_bf,
            "wdeca": wdeca,
            "dechTa": dechTa.astype(bf),
            "vcol": vcol,
            "covf": cov_c,
        }
        if masked:
            m_c = np.ascontiguousarray(mask_f[sl])
            m["emul"] = m_c
            m["eadd"] = ((m_c - 1.0) * 1e9).astype(np.float32)
        in_maps.append(m)

    run = run_bass_kernel_spmd(get_graph(masked), in_maps, list(range(NCORES)))
    global LAST_RUN
    LAST_RUN = run
    res = run.results

    a = np.concatenate([np.asarray(r["o_a"], np.float32) for r in res], axis=0)
    h_star = np.concatenate([np.asarray(r["o_h"], np.float32) for r in res], axis=0)
    new_cov = np.concatenate([np.asarray(r["o_c"], np.float32) for r in res], axis=0)
    covloss = np.concatenate(
        [np.asarray(r["o_l"], np.float32).reshape(BL) for r in res], axis=0
    )
    return (a, h_star, new_cov, covloss)
